# revision 3
# baseline (speedup 1.0000x reference)
"""Causal self-attention (B=2, T=2048, D=1024, H=16) on 8 Trainium2 cores.

Sharding: tensor-parallel over heads x data-parallel over batch.
Core c handles batch b = c // 4 and the 4 heads hg = c % 4 (global heads
4*hg .. 4*hg+3).  Each core computes a partial output (its heads' slice of
the output projection); the host sums the 4 partials per batch.

Device-side layout trick: everything is computed transposed so that no
on-device transposes are needed:
  qT/kT  [m, t]   (m = local head dim, on partitions)
  v      [t, m]   (t on partitions) with a ones column appended per head
  sT     [t_k, t_q] = kT_h.T-contract -> exp -> causal mask via affine_select
  rawT   [65, t_q] = v_aug.T @ exp  (row 64 = softmax denominator)
  ctxT   [64, t_q] = rawT[:64] * (1/denom)  broadcast via a K=1 matmul
  outT   [i, t]   partial projection, summed on host
"""

import sys

sys.path.insert(0, "/opt/trn_rl_repo")

import numpy as np

P = 128
B, T, D = 2, 2048, 1024
H_GLOBAL = 16
HD = 64
M = 256          # local q/k/v dim per core (4 heads x 64)
NH = 4           # local heads
DT = D // P      # 8 d tiles
NT = T // P      # 16 t_k tiles
TB = 512         # t_q block
NB = T // TB     # 4 t_q blocks
MT = M // P      # 2 m tiles
IT = D // P      # 8 output i tiles

N_CORES = 8

_CACHE = {}


def _build_nc(reps: int = 1):
    import concourse.mybir as mybir
    from concourse import bacc
    from concourse.tile import TileContext

    fp32 = mybir.dt.float32
    AF = mybir.ActivationFunctionType

    nc = bacc.Bacc("TRN2", target_bir_lowering=False, debug=False)

    xT = nc.declare_dram_parameter("xT", [P, DT, T], fp32, isOutput=False)
    wqT = nc.declare_dram_parameter("wqT", [P, DT, M], fp32, isOutput=False)
    wkT = nc.declare_dram_parameter("wkT", [P, DT, M], fp32, isOutput=False)
    wvT = nc.declare_dram_parameter("wvT", [P, DT, M], fp32, isOutput=False)
    wpT = nc.declare_dram_parameter("wpT", [P, MT, D], fp32, isOutput=False)
    bq = nc.declare_dram_parameter("bq", [P, MT], fp32, isOutput=False)
    bk = nc.declare_dram_parameter("bk", [P, MT], fp32, isOutput=False)
    bv = nc.declare_dram_parameter("bv", [P, MT], fp32, isOutput=False)
    bp = nc.declare_dram_parameter("bp", [P, IT], fp32, isOutput=False)
    outT = nc.declare_dram_parameter("outT", [D, T], fp32, isOutput=True)

    scale = 1.0 / np.sqrt(HD)

    with TileContext(nc) as tc:
        with (
            tc.tile_pool(name="wconst", bufs=1) as wpool,
            tc.tile_pool(name="xbuf", bufs=1) as xpool,
            tc.tile_pool(name="qkv", bufs=1) as qpool,
            tc.tile_pool(name="exps", bufs=4) as spool,
            tc.tile_pool(name="outs", bufs=3) as opool,
            tc.tile_pool(name="small", bufs=4) as rpool,
            tc.tile_pool(name="psmm", bufs=3, space="PSUM") as psmm,
            tc.tile_pool(name="psacc", bufs=3, space="PSUM") as psacc,
        ):
            # ---- constants ----
            wq_sb = wpool.tile([P, DT, M], fp32, tag="wq")
            wk_sb = wpool.tile([P, DT, M], fp32, tag="wk")
            wv_sb = wpool.tile([P, DT, M], fp32, tag="wv")
            wp_sb = wpool.tile([P, MT, D], fp32, tag="wp")
            bq_sb = wpool.tile([P, MT], fp32, tag="bq")
            bk_sb = wpool.tile([P, MT], fp32, tag="bk")
            bv_sb = wpool.tile([P, MT], fp32, tag="bv")
            bp_sb = wpool.tile([P, IT], fp32, tag="bp")

            nc.sync.dma_start(wq_sb[:], wqT.ap())
            nc.sync.dma_start(wk_sb[:], wkT.ap())
            nc.sync.dma_start(wv_sb[:], wvT.ap())
            nc.sync.dma_start(wp_sb[:], wpT.ap())
            nc.sync.dma_start(bq_sb[:], bq.ap())
            nc.sync.dma_start(bk_sb[:], bk.ap())
            nc.sync.dma_start(bv_sb[:], bv.ap())
            nc.sync.dma_start(bp_sb[:], bp.ap())

            x_sb = xpool.tile([P, DT, T], fp32, tag="x")
            for dt in range(DT):
                nc.sync.dma_start(x_sb[:, dt, :], xT.ap()[:, dt, :])

            for _ in range(reps):
                qT_sb = qpool.tile([P, MT, T], fp32, tag="qT")
                kT_sb = qpool.tile([P, MT, T], fp32, tag="kT")
                # v with ones column: [t_part, tk, head, 65]
                va_sb = qpool.tile([P, NT, NH, HD + 1], fp32, tag="va")
                ca_sb = qpool.tile([P, MT, T], fp32, tag="ca")

                nc.vector.memset(va_sb[:, :, :, HD : HD + 1], 1.0)

                # ---- q^T, k^T projections ----
                for w_sb, b_sb, dst in ((wq_sb, bq_sb, qT_sb), (wk_sb, bk_sb, kT_sb)):
                    for j in range(MT):
                        for tb in range(NB):
                            ps = psmm.tile([P, TB], fp32, tag="mm")
                            for dt in range(DT):
                                nc.tensor.matmul(
                                    ps[:],
                                    w_sb[:, dt, j * P : (j + 1) * P],
                                    x_sb[:, dt, tb * TB : (tb + 1) * TB],
                                    start=(dt == 0),
                                    stop=(dt == DT - 1),
                                )
                            nc.vector.tensor_scalar_add(
                                dst[:, j, tb * TB : (tb + 1) * TB],
                                ps[:],
                                b_sb[:, j : j + 1],
                            )

                # ---- v projection (natural layout [t, m]) ----
                for tt in range(NT):
                    ps = psmm.tile([P, TB], fp32, tag="mm")
                    for dt in range(DT):
                        nc.tensor.matmul(
                            ps[:, :M],
                            x_sb[:, dt, tt * P : (tt + 1) * P],
                            wv_sb[:, dt, :],
                            start=(dt == 0),
                            stop=(dt == DT - 1),
                        )
                    nc.vector.tensor_copy(
                        va_sb[:, tt, :, 0:HD],
                        ps[:, :M].rearrange("p (h d) -> p h d", h=NH),
                    )

                # ---- attention per (t_q block, head) ----
                for tb in range(NB):
                    ntk = 4 * (tb + 1)  # causal: t_k tiles 0 .. 4*tb+3
                    for h in range(NH):
                        jj = h // 2
                        pp = (h % 2) * HD
                        acc = psacc.tile([P, TB], fp32, tag="acc")
                        for tt in range(ntk):
                            ps_s = psmm.tile([P, TB], fp32, tag="mm")
                            nc.tensor.matmul(
                                ps_s[:],
                                kT_sb[pp : pp + HD, jj, tt * P : (tt + 1) * P],
                                qT_sb[pp : pp + HD, jj, tb * TB : (tb + 1) * TB],
                                start=True,
                                stop=True,
                            )
                            exp_t = spool.tile([P, TB], fp32, tag="exp")
                            di = tt - 4 * tb
                            if di < 0:
                                nc.scalar.activation(
                                    exp_t[:], ps_s[:], AF.Exp, scale=scale
                                )
                            else:
                                c0 = di * P
                                if c0 > 0:
                                    nc.vector.memset(exp_t[:, :c0], 0.0)
                                nc.scalar.activation(
                                    exp_t[:, c0:], ps_s[:, c0:], AF.Exp, scale=scale
                                )
                                # keep element iff (free_idx - partition) >= 0
                                nc.gpsimd.affine_select(
                                    out=exp_t[:, c0:],
                                    in_=exp_t[:, c0:],
                                    compare_op=mybir.AluOpType.is_ge,
                                    fill=0.0,
                                    base=0,
                                    pattern=[[1, TB - c0]],
                                    channel_multiplier=-1,
                                )
                            nc.tensor.matmul(
                                acc[: HD + 1, :],
                                va_sb[:, tt, h, :],
                                exp_t[:],
                                start=(tt == 0),
                                stop=(tt == ntk - 1),
                            )
                        # normalize: ctxT = raw * (1/denom), bcast over partitions
                        rec = rpool.tile([1, TB], fp32, tag="rec")
                        nc.vector.reciprocal(rec[:], acc[HD : HD + 1, :])
                        rec_b = spool.tile([HD, TB], fp32, tag="recb")
                        nc.gpsimd.partition_broadcast(rec_b[:], rec[:])
                        nc.vector.tensor_mul(
                            ca_sb[pp : pp + HD, jj, tb * TB : (tb + 1) * TB],
                            acc[0:HD, :],
                            rec_b[:],
                        )
                    # v-bias once per (tb, jj) after both heads of the pair
                    for jj in range(MT):
                        nc.vector.tensor_scalar_add(
                            ca_sb[:, jj, tb * TB : (tb + 1) * TB],
                            ca_sb[:, jj, tb * TB : (tb + 1) * TB],
                            bv_sb[:, jj : jj + 1],
                        )

                # ---- output projection (transposed, partial) ----
                for it in range(IT):
                    for tb in range(NB):
                        ps_o = psmm.tile([P, TB], fp32, tag="mm")
                        for jj in range(MT):
                            nc.tensor.matmul(
                                ps_o[:],
                                wp_sb[:, jj, it * P : (it + 1) * P],
                                ca_sb[:, jj, tb * TB : (tb + 1) * TB],
                                start=(jj == 0),
                                stop=(jj == MT - 1),
                            )
                        ot = opool.tile([P, TB], fp32, tag="ot")
                        nc.vector.tensor_scalar_add(
                            ot[:], ps_o[:], bp_sb[:, it : it + 1]
                        )
                        nc.sync.dma_start(
                            outT.ap()[
                                it * P : (it + 1) * P, tb * TB : (tb + 1) * TB
                            ],
                            ot[:],
                        )

    nc.finalize()
    return nc


def _prep_core_inputs(x, Wq, bq, Wk, bk, Wv, bv, Wp, bp, core):
    b = core // 4
    hg = core % 4
    sl = slice(hg * M, (hg + 1) * M)

    def part_inner(a2d):  # [D, F] -> [P, D//P, F]
        return np.ascontiguousarray(
            a2d.reshape(a2d.shape[0] // P, P, a2d.shape[1]).transpose(1, 0, 2)
        )

    xT = part_inner(np.ascontiguousarray(x[b].T))           # [128, 8, 2048]
    wq = part_inner(np.ascontiguousarray(Wq[sl].T))         # [128, 8, 256]
    wk = part_inner(np.ascontiguousarray(Wk[sl].T))
    wv = part_inner(np.ascontiguousarray(Wv[sl].T))
    wp = part_inner(np.ascontiguousarray(Wp[:, sl].T))      # [128, 2, 1024]
    return {
        "xT": xT,
        "wqT": wq,
        "wkT": wk,
        "wvT": wv,
        "wpT": wp,
        "bq": np.ascontiguousarray(bq[sl].reshape(MT, P).T),
        "bk": np.ascontiguousarray(bk[sl].reshape(MT, P).T),
        "bv": np.ascontiguousarray(bv[sl].reshape(MT, P).T),
        "bp": np.ascontiguousarray(bp.reshape(IT, P).T),
    }


def kernel(x, Wq, bq, Wk, bk, Wv, bv, Wp, bp):
    x = np.asarray(x, dtype=np.float32)
    Wq = np.asarray(Wq, dtype=np.float32)
    Wk = np.asarray(Wk, dtype=np.float32)
    Wv = np.asarray(Wv, dtype=np.float32)
    Wp = np.asarray(Wp, dtype=np.float32)
    bq = np.asarray(bq, dtype=np.float32)
    bk = np.asarray(bk, dtype=np.float32)
    bv = np.asarray(bv, dtype=np.float32)
    bp = np.asarray(bp, dtype=np.float32)

    if "nc" not in _CACHE:
        _CACHE["nc"] = _build_nc()
    nc = _CACHE["nc"]

    from concourse.bass_utils import run_bass_kernel_spmd

    in_maps = [
        _prep_core_inputs(x, Wq, bq, Wk, bk, Wv, bv, Wp, bp, c)
        for c in range(N_CORES)
    ]
    res = run_bass_kernel_spmd(nc, in_maps, list(range(N_CORES)))

    out = np.zeros((B, T, D), dtype=np.float32)
    for c in range(N_CORES):
        out[c // 4] += res.results[c]["outT"].T
    return out


# revision 6
# speedup vs baseline: 3.7755x; 3.7755x over previous
"""Causal self-attention (B=2, T=2048, D=1024, H=16) on 8 Trainium2 cores.

Sharding: tensor-parallel over heads x data-parallel over batch.
Core c handles batch b = c // 4 and the 4 heads hg = c % 4 (global heads
4*hg .. 4*hg+3).  Each core computes a partial output (its heads' slice of
the output projection); the host sums the 4 partials per batch.

Device-side layout: everything is computed transposed so that no on-device
transposes are needed:
  qT/kT  [m, t]   (m = local head dim, on partitions), fp16
  v      [t, m]   (t on partitions) with a ones column appended per head, fp16
  sT     [t_k, t_q] = kT_h x qT_h -> exp (fp16) -> causal mask (affine_select)
  rawT   [65, t_q] = v_aug.T @ exp in fp32 PSUM (row 64 = softmax denominator)
  ctxT   [64, t_q] = rawT[:64] * (1/denom), denom broadcast on GPSIMD
  outT   [i, t]   partial projection in fp32, summed/transposed on host

Matmul operands are fp16 (PE runs fp32 matmuls at 1/4 rate); all
accumulation stays in fp32 PSUM.
"""

import sys

sys.path.insert(0, "/opt/trn_rl_repo")

import numpy as np

P = 128
B, T, D = 2, 2048, 1024
HD = 64
M = 256          # local q/k/v dim per core (4 heads x 64)
NH = 4           # local heads
DT = D // P      # 8 d tiles
NT = T // P      # 16 t_k tiles
TB = 512         # t_q block
NB = T // TB     # 4 t_q blocks
MT = M // P      # 2 m tiles
IT = D // P      # 8 output i tiles

N_CORES = 8

_CACHE = {}


def _build_nc(reps: int = 1, loop_n: int = 0):
    import contextlib

    import concourse.mybir as mybir
    from concourse import bacc
    from concourse.tile import TileContext

    fp32 = mybir.dt.float32
    fp16 = mybir.dt.float16
    AF = mybir.ActivationFunctionType

    nc = bacc.Bacc("TRN2", target_bir_lowering=False, debug=False)

    xT = nc.declare_dram_parameter("xT", [P, DT, T], fp16, isOutput=False)
    wqT = nc.declare_dram_parameter("wqT", [P, DT, M], fp16, isOutput=False)
    wkT = nc.declare_dram_parameter("wkT", [P, DT, M], fp16, isOutput=False)
    wvT = nc.declare_dram_parameter("wvT", [P, DT, M], fp16, isOutput=False)
    wpT = nc.declare_dram_parameter("wpT", [P, MT, D], fp16, isOutput=False)
    bq = nc.declare_dram_parameter("bq", [P, MT], fp32, isOutput=False)
    bk = nc.declare_dram_parameter("bk", [P, MT], fp32, isOutput=False)
    bv = nc.declare_dram_parameter("bv", [P, MT], fp32, isOutput=False)
    bp = nc.declare_dram_parameter("bp", [P, IT], fp32, isOutput=False)
    outT = nc.declare_dram_parameter("outT", [D, T], fp32, isOutput=True)

    scale = 1.0 / np.sqrt(HD)

    with TileContext(nc) as tc:
        with (
            tc.tile_pool(name="wconst", bufs=1) as wpool,
            tc.tile_pool(name="xbuf", bufs=1) as xpool,
            tc.tile_pool(name="qkv", bufs=1) as qpool,
            tc.tile_pool(name="exps", bufs=4) as spool,
            tc.tile_pool(name="outs", bufs=3) as opool,
            tc.tile_pool(name="small", bufs=4) as rpool,
            tc.tile_pool(name="psmm", bufs=3, space="PSUM") as psmm,
            tc.tile_pool(name="psacc", bufs=2, space="PSUM") as psacc,
        ):
            # ---- constants ----
            wq_sb = wpool.tile([P, DT, M], fp16, tag="wq")
            wk_sb = wpool.tile([P, DT, M], fp16, tag="wk")
            wv_sb = wpool.tile([P, DT, M], fp16, tag="wv")
            wp_sb = wpool.tile([P, MT, D], fp16, tag="wp")
            bq_sb = wpool.tile([P, MT], fp32, tag="bq")
            bk_sb = wpool.tile([P, MT], fp32, tag="bk")
            bv_sb = wpool.tile([P, MT], fp32, tag="bv")
            bp_sb = wpool.tile([P, IT], fp32, tag="bp")

            nc.sync.dma_start(wq_sb[:], wqT.ap())
            nc.sync.dma_start(wk_sb[:], wkT.ap())
            nc.sync.dma_start(wv_sb[:], wvT.ap())
            nc.sync.dma_start(wp_sb[:], wpT.ap())
            nc.sync.dma_start(bq_sb[:], bq.ap())
            nc.sync.dma_start(bk_sb[:], bk.ap())
            nc.sync.dma_start(bv_sb[:], bv.ap())
            nc.sync.dma_start(bp_sb[:], bp.ap())

            x_sb = xpool.tile([P, DT, T], fp16, tag="x")
            for dt in range(DT):
                nc.sync.dma_start(x_sb[:, dt, :], xT.ap()[:, dt, :])

            if loop_n:
                loop_cm = tc.For_i(
                    0, loop_n, 1,
                    hint_engines=(
                        mybir.EngineType.PE,
                        mybir.EngineType.Activation,
                        mybir.EngineType.DVE,
                        mybir.EngineType.Pool,
                        mybir.EngineType.SP,
                    ),
                )
            else:
                loop_cm = contextlib.nullcontext()
            with loop_cm:
              for _ in range(reps):
                qT_sb = qpool.tile([P, MT, T], fp16, tag="qT")
                kT_sb = qpool.tile([P, MT, T], fp16, tag="kT")
                # v with ones column: [t_part, tk, head, 65]
                va_sb = qpool.tile([P, NT, NH, HD + 1], fp16, tag="va")
                ca_sb = qpool.tile([P, MT, T], fp16, tag="ca")

                nc.vector.memset(va_sb[:, :, :, HD : HD + 1], 1.0)

                # ---- q^T, k^T projections (two 512-blocks per PSUM tile) ----
                for w_sb, b_sb, dst in ((wq_sb, bq_sb, qT_sb), (wk_sb, bk_sb, kT_sb)):
                    for j in range(MT):
                        for tbp in range(NB // 2):
                            ps = psmm.tile([P, 2 * TB], fp32, tag="mm")
                            for half in range(2):
                                tb = 2 * tbp + half
                                for dt in range(DT):
                                    nc.tensor.matmul(
                                        ps[:, half * TB : (half + 1) * TB],
                                        w_sb[:, dt, j * P : (j + 1) * P],
                                        x_sb[:, dt, tb * TB : (tb + 1) * TB],
                                        start=(dt == 0),
                                        stop=(dt == DT - 1),
                                    )
                            nc.vector.tensor_scalar_add(
                                dst[:, j, tbp * 2 * TB : (tbp + 1) * 2 * TB],
                                ps[:],
                                b_sb[:, j : j + 1],
                            )

                # ---- v projection (natural layout [t, m]; 4 t-tiles/PSUM) ----
                for tq in range(NT // 4):
                    ps = psmm.tile([P, 2 * TB], fp32, tag="mm")
                    for q4 in range(4):
                        tt = 4 * tq + q4
                        for dt in range(DT):
                            nc.tensor.matmul(
                                ps[:, q4 * M : (q4 + 1) * M],
                                x_sb[:, dt, tt * P : (tt + 1) * P],
                                wv_sb[:, dt, :],
                                start=(dt == 0),
                                stop=(dt == DT - 1),
                            )
                    nc.vector.tensor_copy(
                        va_sb[:, 4 * tq : 4 * tq + 4, :, 0:HD],
                        ps[:].rearrange("p (t h d) -> p t h d", t=4, h=NH),
                    )

                # ---- attention per (t_q block, head) ----
                for tb in range(NB):
                    ntk = 4 * (tb + 1)  # causal: t_k tiles 0 .. 4*tb+3
                    for h in range(NH):
                        jj = h // 2
                        pp = (h % 2) * HD
                        acc = psacc.tile([P, TB], fp32, tag="acc")
                        for tp in range(ntk // 2):
                            ps_s = psmm.tile([P, 2 * TB], fp32, tag="mm")
                            exp_t = spool.tile([P, 2 * TB], fp16, tag="exp")
                            for half in range(2):
                                tt = 2 * tp + half
                                nc.tensor.matmul(
                                    ps_s[:, half * TB : (half + 1) * TB],
                                    kT_sb[pp : pp + HD, jj, tt * P : (tt + 1) * P],
                                    qT_sb[pp : pp + HD, jj, tb * TB : (tb + 1) * TB],
                                    start=True,
                                    stop=True,
                                )
                            if 2 * tp + 1 < 4 * tb:
                                # both halves fully below the diagonal
                                nc.scalar.activation(
                                    exp_t[:], ps_s[:], AF.Exp, scale=scale
                                )
                            else:
                                for half in range(2):
                                    tt = 2 * tp + half
                                    di = tt - 4 * tb
                                    lo = half * TB
                                    hi = (half + 1) * TB
                                    if di < 0:
                                        nc.scalar.activation(
                                            exp_t[:, lo:hi], ps_s[:, lo:hi],
                                            AF.Exp, scale=scale,
                                        )
                                        continue
                                    c0 = di * P
                                    if c0 > 0:
                                        nc.vector.memset(exp_t[:, lo : lo + c0], 0.0)
                                    nc.scalar.activation(
                                        exp_t[:, lo + c0 : hi],
                                        ps_s[:, lo + c0 : hi],
                                        AF.Exp, scale=scale,
                                    )
                                    # keep element iff (free_idx - partition) >= 0
                                    nc.gpsimd.affine_select(
                                        out=exp_t[:, lo + c0 : hi],
                                        in_=exp_t[:, lo + c0 : hi],
                                        compare_op=mybir.AluOpType.is_ge,
                                        fill=0.0,
                                        base=0,
                                        pattern=[[1, TB - c0]],
                                        channel_multiplier=-1,
                                    )
                            for half in range(2):
                                tt = 2 * tp + half
                                nc.tensor.matmul(
                                    acc[: HD + 1, :],
                                    va_sb[:, tt, h, :],
                                    exp_t[:, half * TB : (half + 1) * TB],
                                    start=(tt == 0),
                                    stop=(tt == ntk - 1),
                                )
                        # normalize: ctxT = raw * (1/denom), bcast over partitions
                        rec = rpool.tile([1, TB], fp32, tag="rec")
                        nc.vector.reciprocal(rec[:], acc[HD : HD + 1, :])
                        rec_b = rpool.tile([HD, TB], fp32, tag="recb")
                        nc.gpsimd.partition_broadcast(rec_b[:], rec[:])
                        nc.vector.tensor_mul(
                            ca_sb[pp : pp + HD, jj, tb * TB : (tb + 1) * TB],
                            acc[0:HD, :],
                            rec_b[:],
                        )
                    # v-bias once per (tb, jj) after both heads of the pair
                    for jj in range(MT):
                        nc.vector.tensor_scalar_add(
                            ca_sb[:, jj, tb * TB : (tb + 1) * TB],
                            ca_sb[:, jj, tb * TB : (tb + 1) * TB],
                            bv_sb[:, jj : jj + 1],
                        )

                # ---- output projection (transposed, partial) ----
                for it in range(IT):
                    for tbp in range(NB // 2):
                        ps_o = psmm.tile([P, 2 * TB], fp32, tag="mm")
                        for half in range(2):
                            tb = 2 * tbp + half
                            for jj in range(MT):
                                nc.tensor.matmul(
                                    ps_o[:, half * TB : (half + 1) * TB],
                                    wp_sb[:, jj, it * P : (it + 1) * P],
                                    ca_sb[:, jj, tb * TB : (tb + 1) * TB],
                                    start=(jj == 0),
                                    stop=(jj == MT - 1),
                                )
                        ot = opool.tile([P, 2 * TB], fp32, tag="ot")
                        nc.vector.tensor_scalar_add(
                            ot[:], ps_o[:], bp_sb[:, it : it + 1]
                        )
                        nc.sync.dma_start(
                            outT.ap()[
                                it * P : (it + 1) * P,
                                tbp * 2 * TB : (tbp + 1) * 2 * TB,
                            ],
                            ot[:],
                        )

    nc.finalize()
    return nc


def _prep_core_inputs(x, Wq, bq, Wk, bk, Wv, bv, Wp, bp, core):
    b = core // 4
    hg = core % 4
    sl = slice(hg * M, (hg + 1) * M)

    def part_inner(a2d):  # [D, F] -> [P, D//P, F], cast to fp16
        a = a2d.reshape(a2d.shape[0] // P, P, a2d.shape[1]).transpose(1, 0, 2)
        return np.ascontiguousarray(a.astype(np.float16))

    return {
        "xT": part_inner(x[b].T),                       # [128, 8, 2048]
        "wqT": part_inner(Wq[sl].T),                    # [128, 8, 256]
        "wkT": part_inner(Wk[sl].T),
        "wvT": part_inner(Wv[sl].T),
        "wpT": part_inner(Wp[:, sl].T),                 # [128, 2, 1024]
        "bq": np.ascontiguousarray(bq[sl].reshape(MT, P).T),
        "bk": np.ascontiguousarray(bk[sl].reshape(MT, P).T),
        "bv": np.ascontiguousarray(bv[sl].reshape(MT, P).T),
        "bp": np.ascontiguousarray(bp.reshape(IT, P).T),
    }


def kernel(x, Wq, bq, Wk, bk, Wv, bv, Wp, bp):
    x = np.asarray(x, dtype=np.float32)
    Wq = np.asarray(Wq, dtype=np.float32)
    Wk = np.asarray(Wk, dtype=np.float32)
    Wv = np.asarray(Wv, dtype=np.float32)
    Wp = np.asarray(Wp, dtype=np.float32)
    bq = np.asarray(bq, dtype=np.float32)
    bk = np.asarray(bk, dtype=np.float32)
    bv = np.asarray(bv, dtype=np.float32)
    bp = np.asarray(bp, dtype=np.float32)

    if "nc" not in _CACHE:
        _CACHE["nc"] = _build_nc()
    nc = _CACHE["nc"]

    from concourse.bass_utils import run_bass_kernel_spmd

    in_maps = [
        _prep_core_inputs(x, Wq, bq, Wk, bk, Wv, bv, Wp, bp, c)
        for c in range(N_CORES)
    ]
    res = run_bass_kernel_spmd(nc, in_maps, list(range(N_CORES)))

    out = np.zeros((B, T, D), dtype=np.float32)
    for c in range(N_CORES):
        out[c // 4] += res.results[c]["outT"].T
    return out
